# revision 1
# baseline (speedup 1.0000x reference)
"""CRF-RNN layer (nn_CrfRnnLayer) as an 8-core Trainium2 Bass kernel.

Distribution (sharding_hint): shard the N=H*W pixel dimension across the 8
cores; each core owns a strip of S=N/8=800 output pixels and holds the
(N x S) slices of both Gaussian kernels fully resident in SBUF as fp8.

Phase 0 builds the kernel slices on-device:
  * bilateral: one fp16 matmul per 128-pixel j-tile using a hi/lo split of
    the augmented features g=[f,1,-|f|^2/2], h=[f,-|f|^2/2,1] stacked as
    [g_hi;g_hi;g_lo] . [h_hi;h_lo;h_hi] (21-dim contraction) -> exact to
    ~1e-2 in d^2 at full bf16-class PE speed; Exp on the scalar engine
    writes fp8 tiles straight into SBUF.
  * spatial: exact integer arithmetic in fp16 via
    g=[x'^2,y'^2,1,1,2x',2y'], h=[1,1,x'^2,y'^2,-x',-y'] with per-core
    centered coordinates, Exp(scale=-1/18).
  * norms Sum_j K[j,i] via fp8 DoubleRow matmuls against ones, transposed
    into per-pixel-block scalars, negated reciprocals cached.

Phase 1 runs the 5 mean-field iterations with q kept in (pixel, class)
layout: local softmax -> fp8 AllGather of softmax (padded to 32 cols so
every DMA is contiguous) -> fp8 DoubleRow filter matmuls against the
SBUF-resident kernels (norm folded in afterwards as per-partition scalars)
-> compatibility fold via small matmuls (compat pre-multiplied into the
kernel-weight matrices on the host) -> q update.  The first iteration's
softmax+AllGather is issued before phase 0 so the collective overlaps the
kernel build.
"""

import json

import numpy as np

from concourse import bacc, bass, mybir, tile
from concourse.bass_utils import run_bass_kernel_spmd

H = W = 80
C = 21
CP = 32              # padded class dim (fp8 DoubleRow needs 16B-aligned strides)
N = H * W            # 6400
M = 8                # cores
S = N // M           # 800 pixels per strip
NIT = 5
NS2 = N // 256       # 25 super-tiles of 256 pixels (DoubleRow pairs)
THETA_ALPHA, THETA_BETA, THETA_GAMMA = 160.0, 3.0, 3.0
F32 = mybir.dt.float32
F16 = mybir.dt.float16
FP8 = mybir.dt.float8e4
H1 = 512             # psum-bank split of the 800-wide strip

_CACHE = {}


def _split_bir_multiwaits(bir_json: bytes) -> bytes:
    """Split >1-sync-wait instructions into single-wait chains.

    The staged walrus build allows only one embedded sync-wait per
    instruction; prepend pure-wait EventSemaphores (same engine, same
    block) for all but the last wait.  Tile completion semaphores only
    count up within the kernel epoch, so waiting sequentially is
    equivalent to the simultaneous multi-wait.
    """
    d = json.loads(bir_json)
    for fn in d.get("functions", []):
        for blk in fn.get("blocks", []):
            out = []
            for inst in blk.get("instructions", []):
                si = inst.get("sync_info") or {}
                waits = si.get("on_wait") or []
                if len(waits) > 1:
                    for j, w in enumerate(waits[:-1]):
                        out.append({
                            "debug": inst.get("debug", 0),
                            "engine": inst["engine"],
                            "ins": [],
                            "name": f"{inst['name']}-sw{j}",
                            "opcode": "EventSemaphore",
                            "outs": [],
                            "sync_info": {"on_update": [], "on_wait": [w]},
                        })
                    si["on_wait"] = [waits[-1]]
                out.append(inst)
            blk["instructions"] = out
    return json.dumps(d).encode()


def _install_birpatch():
    if _CACHE.get("birpatch"):
        return
    from concourse import bass2jax
    orig = bass2jax.compile_bir_kernel

    def patched(bir_json, tmpdir, neff_name="file.neff"):
        return orig(_split_bir_multiwaits(bir_json), tmpdir, neff_name)

    bass2jax.compile_bir_kernel = patched
    _CACHE["birpatch"] = True


def _build_program(nit=NIT):
    key = ("nc", nit)
    if key in _CACHE:
        return _CACHE[key]
    nc = bacc.Bacc("TRN2", target_bir_lowering=False, debug=False, num_devices=M)

    gbT = nc.dram_tensor("gbT", [C, N], F16, kind="ExternalInput")
    hbT = nc.dram_tensor("hbT", [C, S], F16, kind="ExternalInput")
    gsT = nc.dram_tensor("gsT", [6, N], F16, kind="ExternalInput")
    hsT = nc.dram_tensor("hsT", [6, S], F16, kind="ExternalInput")
    u_px = nc.dram_tensor("u_px", [100, 8, C], F32, kind="ExternalInput")
    skbkT = nc.dram_tensor("skbkT", [53, C], F16, kind="ExternalInput")
    sm1 = nc.dram_tensor("sm1", [N, CP], FP8, kind="ExternalInput")
    rscT = nc.dram_tensor("rscT", [C, S], F32, kind="ExternalInput")
    q_out = nc.dram_tensor("q_out", [100, 8, C], F32, kind="ExternalOutput")

    EXP = mybir.ActivationFunctionType.Exp
    COPY = mybir.ActivationFunctionType.Copy
    DR = mybir.MatmulPerfMode.DoubleRow

    with tile.TileContext(nc) as tc:
        with (
            tc.tile_pool(name="const", bufs=1) as constp,
            tc.tile_pool(name="smtile", bufs=2) as smtp,
            tc.tile_pool(name="smfull", bufs=2) as smfp,
            tc.tile_pool(name="fcopy", bufs=2) as fcp,
            tc.tile_pool(name="qpool", bufs=2) as qp,
            tc.tile_pool(name="dram_cc", bufs=2, space="DRAM") as dramcc,
        ):
            # ---- resident constants ----
            gb_sb = constp.tile([C, N], F16, tag="gb")
            nc.sync.dma_start(gb_sb[:], gbT[:, :])
            hb_sb = constp.tile([C, S], F16, tag="hb")
            nc.sync.dma_start(hb_sb[:], hbT[:, :])
            gs_sb = constp.tile([6, N], F16, tag="gs")
            nc.sync.dma_start(gs_sb[:], gsT[:, :])
            hs_sb = constp.tile([6, S], F16, tag="hs")
            nc.sync.dma_start(hs_sb[:], hsT[:, :])
            u_sb = constp.tile([100, 8, C], F32, tag="u")
            nc.sync.dma_start(u_sb[:], u_px[:, :, :])
            skbk_sb = constp.tile([53, C], F16, tag="skbk")
            nc.sync.dma_start(skbk_sb[:], skbkT[:, :])
            ones2 = constp.tile([128, 2, 16], FP8, tag="ones2")
            nc.vector.memset(ones2[:], 1.0)
            onesC = constp.tile([1, C], F16, tag="onesC")
            nc.vector.memset(onesC[:], 1.0)

            kb_sb = constp.tile([128, NS2, 2, S], FP8, tag="kb")
            ks_sb = constp.tile([128, NS2, 2, S], FP8, tag="ks")
            # broadcast -1/norm rows, (C, S) per kernel
            rbc_sb = constp.tile([C, S], F32, tag="rbc")
            rsc_sb = constp.tile([C, S], F32, tag="rsc")
            nc.sync.dma_start(rsc_sb[:], rscT[:, :])

            X = mybir.AxisListType.X
            ADD = mybir.AluOpType.add

            def softmax_and_gather(q_tile):
                smcat = smtp.tile([100, 8, CP], FP8, tag="smcat")
                nc.vector.memset(smcat[:, :, C:CP], 0.0)
                esb = smtp.tile([100, 8, C], F32, tag="esb")
                nc.scalar.activation(esb[:], q_tile[:], EXP)
                ssum = smtp.tile([100, 8], F32, tag="ssum")
                nc.vector.tensor_reduce(ssum[:], esb[:], X, ADD)
                rsum = smtp.tile([100, 8], F32, tag="rsum")
                nc.vector.reciprocal(rsum[:], ssum[:])
                for s8 in range(8):
                    nc.vector.tensor_scalar_mul(
                        smcat[:, s8, 0:C], esb[:, s8, :], rsum[:, s8:s8 + 1])
                sm_in = dramcc.tile([S, CP], FP8, tag="sm_in")
                nc.sync.dma_start(
                    sm_in[:, :].rearrange("(s p) c -> p s c", p=100), smcat[:])
                sm_all = dramcc.tile([N, CP], FP8, tag="sm_all")
                nc.gpsimd.collective_compute(
                    "AllGather",
                    mybir.AluOpType.bypass,
                    replica_groups=[list(range(M))],
                    ins=[sm_in[:, :].opt()],
                    outs=[sm_all[:, :].opt()],
                )
                return sm_all

            sm_all = None  # iteration 1 reads the host-computed sm1 input

            # ---- phase 0: materialize fp8 kernel slices in SBUF + norms ----
            with (
                tc.tile_pool(name="psum_ip", bufs=2, space="PSUM") as pip,
                tc.tile_pool(name="psum_norm", bufs=1, space="PSUM") as pnorm,
                tc.tile_pool(name="psum_bc", bufs=1, space="PSUM") as pbc,
            ):
                for g_sb, h_sb, k_sb, scale, with_norm in (
                    (gb_sb, hb_sb, kb_sb, 1.0, True),
                    (gs_sb, hs_sb, ks_sb, -1.0 / 18.0, False),
                ):
                    if with_norm:
                        norm_ps = pnorm.tile([1, S], F32, tag="norm")
                    for st in range(NS2):
                        for t2 in range(2):
                            T = st * 2 + t2
                            ip = pip.tile([128, S], F32, tag="ip")
                            nc.tensor.matmul(
                                ip[:, 0:H1],
                                lhsT=g_sb[:, T * 128:(T + 1) * 128],
                                rhs=h_sb[:, 0:H1], start=True, stop=True)
                            nc.tensor.matmul(
                                ip[:, H1:S],
                                lhsT=g_sb[:, T * 128:(T + 1) * 128],
                                rhs=h_sb[:, H1:S], start=True, stop=True)
                            nc.scalar.activation(
                                k_sb[:, st, t2, :], ip[:, :], EXP, scale=scale)
                        if with_norm:
                            # norm accumulation rides the PE gaps behind Exp
                            nc.tensor.matmul(
                                norm_ps[0:1, 0:H1], lhsT=ones2[:, :, 0:1],
                                rhs=k_sb[:, st, :, 0:H1],
                                start=(st == 0), stop=(st == NS2 - 1),
                                perf_mode=DR)
                            nc.tensor.matmul(
                                norm_ps[0:1, H1:S], lhsT=ones2[:, :, 0:1],
                                rhs=k_sb[:, st, :, H1:S],
                                start=(st == 0), stop=(st == NS2 - 1),
                                perf_mode=DR)
                    if not with_norm:
                        continue
                    # -1/norm broadcast to all C partitions:
                    # reciprocal -> negate (fp16) -> K=1 matmul broadcast
                    nr_sb = smtp.tile([1, S], F32, tag="nr")
                    nc.vector.reciprocal(nr_sb[:], norm_ps[0:1, :])
                    nr16 = smtp.tile([1, S], F16, tag="nr16")
                    nc.scalar.activation(nr16[:], nr_sb[:], COPY, scale=-1.0)
                    bc_ps = pbc.tile([C, S], F32, tag="bc")
                    nc.tensor.matmul(bc_ps[:, 0:H1], lhsT=onesC[:],
                                     rhs=nr16[0:1, 0:H1], start=True, stop=True)
                    nc.tensor.matmul(bc_ps[:, H1:S], lhsT=onesC[:],
                                     rhs=nr16[0:1, H1:S], start=True, stop=True)
                    nc.vector.tensor_copy(rbc_sb[:], bc_ps[:, :])

            # ---- phase 1: mean-field iterations ----
            with (
                tc.tile_pool(name="psum_acc", bufs=1, space="PSUM") as pacc,
                tc.tile_pool(name="psum_pw", bufs=2, space="PSUM") as ppw,
            ):
                q_cur = u_sb
                for it in range(nit):
                    src_ap = sm1 if it == 0 else sm_all
                    smf = smfp.tile([128, NS2, 2, CP], FP8, tag="smf")
                    nc.sync.dma_start(
                        smf[:],
                        src_ap[:, :].rearrange("(p s t) c -> p s t c",
                                               s=NS2, t=2))
                    psb = pacc.tile([C, S], F32, tag="psb")
                    pss = pacc.tile([C, S], F32, tag="pss")
                    for st in range(NS2):
                        lhs = smf[:, st, :, 0:C]
                        st_f, sp_f = (st == 0), (st == NS2 - 1)
                        nc.tensor.matmul(psb[:, 0:H1], lhsT=lhs,
                                         rhs=kb_sb[:, st, :, 0:H1],
                                         start=st_f, stop=sp_f, perf_mode=DR)
                        nc.tensor.matmul(psb[:, H1:S], lhsT=lhs,
                                         rhs=kb_sb[:, st, :, H1:S],
                                         start=st_f, stop=sp_f, perf_mode=DR)
                        nc.tensor.matmul(pss[:, 0:H1], lhsT=lhs,
                                         rhs=ks_sb[:, st, :, 0:H1],
                                         start=st_f, stop=sp_f, perf_mode=DR)
                        nc.tensor.matmul(pss[:, H1:S], lhsT=lhs,
                                         rhs=ks_sb[:, st, :, H1:S],
                                         start=st_f, stop=sp_f, perf_mode=DR)
                    # normalized filter outputs -> stacked fp16 pairwise lhsT:
                    # rows 0:21 spatial*(-1/norm_s), rows 21:42 bilateral*(-1/norm_b)
                    fsb = fcp.tile([53, S], F16, tag="fsb")
                    nc.vector.memset(fsb[0:32, :], 0.0)
                    nc.vector.tensor_mul(fsb[0:C, 0:H1], pss[:, 0:H1],
                                         rsc_sb[:, 0:H1])
                    nc.vector.tensor_mul(fsb[0:C, H1:S], pss[:, H1:S],
                                         rsc_sb[:, H1:S])
                    nc.vector.tensor_mul(fsb[32:53, 0:H1], psb[:, 0:H1],
                                         rbc_sb[:, 0:H1])
                    nc.vector.tensor_mul(fsb[32:53, H1:S], psb[:, H1:S],
                                         rbc_sb[:, H1:S])
                    # pairwise fold: one 42-dim matmul per pixel block
                    pw = ppw.tile([100, 8, C], F32, tag="pw")
                    for s8 in range(8):
                        sl = slice(s8 * 100, (s8 + 1) * 100)
                        nc.tensor.matmul(pw[:, s8, :], lhsT=fsb[:, sl],
                                         rhs=skbk_sb[:], start=True, stop=True)
                    qn = qp.tile([100, 8, C], F32, tag="qn")
                    nc.vector.tensor_add(qn[:], u_sb[:], pw[:, :, :])
                    q_cur = qn
                    if it < nit - 1:
                        sm_all = softmax_and_gather(qn)
                nc.sync.dma_start(q_out[:, :, :], q_cur[:])

    nc.compile()
    _CACHE[key] = nc
    return nc


def _host_prep(unaries, rgb, spatial_kernel, bilateral_kernel,
               compatibility_matrix):
    unaries = np.ascontiguousarray(unaries, dtype=np.float32)
    rgb = np.ascontiguousarray(rgb, dtype=np.float32)
    sk = np.asarray(spatial_kernel, dtype=np.float32)
    bk = np.asarray(bilateral_kernel, dtype=np.float32)
    cm = np.asarray(compatibility_matrix, dtype=np.float32)

    ys, xs = np.meshgrid(np.arange(H, dtype=np.float64),
                         np.arange(W, dtype=np.float64), indexing="ij")
    xs, ys = xs.ravel(), ys.ravel()                     # (N,) pixel coords
    img = rgb[0].reshape(N, 3).astype(np.float64)

    # bilateral: hi/lo fp16 split of augmented features
    fb = np.concatenate([xs[:, None] / THETA_ALPHA, ys[:, None] / THETA_ALPHA,
                         img / THETA_BETA], axis=1)     # (N, 5) f64
    sq = 0.5 * (fb * fb).sum(axis=1)
    onesN = np.ones((N, 1))
    g7 = np.concatenate([fb, onesN, -sq[:, None]], axis=1)   # (N, 7)
    h7 = np.concatenate([fb, -sq[:, None], onesN], axis=1)   # (N, 7)

    def split(a):
        hi = a.astype(np.float16)
        lo = (a - hi.astype(np.float64)).astype(np.float16)
        return hi, lo

    g_hi, g_lo = split(g7)
    h_hi, h_lo = split(h7)
    gb21 = np.concatenate([g_hi, g_hi, g_lo], axis=1)   # (N, 21)
    hb21 = np.concatenate([h_hi, h_lo, h_hi], axis=1)   # (N, 21)
    gbT = np.ascontiguousarray(gb21.T)                  # (21, N) f16

    # spatial: exact integer features, per-core centered y
    # partition-interleaved strips: core d owns pixels with
    # (px % 128) in [16d, 16d+16); local index o = (px%128 - 16d)*50 + px//128.
    # Gathered softmax rows are then p-major: row = p*50 + jtile, so the
    # per-iteration smf DMA is fully contiguous.
    jt_of_o = np.arange(S) % 50
    pp_of_o = np.arange(S) // 50
    xi = xs - 40.0                                      # |x'| <= 40
    yi_g = ys - 40.0                                    # |y'| <= 40
    u_cn = unaries[0].reshape(N, C)
    skbkT = np.ascontiguousarray(np.concatenate(
        [(cm @ sk).T, np.zeros((11, C), np.float32), (cm @ bk).T],
        axis=0).astype(np.float16))                     # (53, 21), zero gap

    # iteration-1 softmax (q_1 = u) computed host-side, fp8-quantized
    import ml_dtypes
    e = np.exp(u_cn.astype(np.float64)
               - u_cn.max(axis=1, keepdims=True).astype(np.float64))
    smx = (e / e.sum(axis=1, keepdims=True)).astype(np.float32)
    sm1 = np.zeros((N, CP), ml_dtypes.float8_e4m3)
    sm1[:, 0:C] = smx.astype(ml_dtypes.float8_e4m3)

    # exact spatial norm row (input-independent): separable full 1D sums
    dx = np.arange(W, dtype=np.float64)
    kx1 = np.exp(-((dx[:, None] - dx[None, :]) ** 2) / 18.0)  # (W, W)
    rowsum_x = kx1.sum(axis=1)                                # (W,)
    dyv = np.arange(H, dtype=np.float64)
    ky1 = np.exp(-((dyv[:, None] - dyv[None, :]) ** 2) / 18.0)
    rowsum_y = ky1.sum(axis=1)                                # (H,)
    norm_s = (rowsum_y[:, None] * rowsum_x[None, :]).ravel()  # (N,)
    rsc_neg = (-1.0 / norm_s).astype(np.float32)              # (N,)

    gs6 = np.stack([xi * xi, yi_g * yi_g, np.ones(N), np.ones(N),
                    2.0 * xi, 2.0 * yi_g], axis=0).astype(np.float16)
    # sm1 in gathered (p-major) layout: row r <-> global px (r%50)*128 + r//50
    r = np.arange(N)
    sm1_g = np.ascontiguousarray(sm1[(r % 50) * 128 + r // 50])

    in_maps = []
    for d in range(M):
        gidx = jt_of_o * 128 + 16 * d + pp_of_o         # local o -> global px
        hs6 = np.stack([np.ones(S), np.ones(S),
                        (xi * xi)[gidx], (yi_g * yi_g)[gidx],
                        -xi[gidx], -yi_g[gidx]], axis=0).astype(np.float16)
        u_strip = u_cn[gidx].reshape(8, 100, C).transpose(1, 0, 2)
        in_maps.append({
            "gbT": gbT,
            "hbT": np.ascontiguousarray(hb21[gidx].T),
            "gsT": np.ascontiguousarray(gs6),
            "hsT": np.ascontiguousarray(hs6),
            "u_px": np.ascontiguousarray(u_strip),
            "skbkT": skbkT,
            "sm1": sm1_g,
            "rscT": np.ascontiguousarray(
                np.broadcast_to(rsc_neg[gidx][None, :], (C, S))),
        })
    return in_maps


def kernel(unaries, rgb, spatial_kernel, bilateral_kernel,
           compatibility_matrix, _run_kwargs=None):
    _install_birpatch()
    nc = _build_program()
    in_maps = _host_prep(unaries, rgb, spatial_kernel, bilateral_kernel,
                         compatibility_matrix)
    kwargs = dict(_run_kwargs or {})
    res = run_bass_kernel_spmd(nc, in_maps, core_ids=list(range(M)), **kwargs)
    _CACHE["last_results"] = res
    jt_of_o = np.arange(S) % 50
    pp_of_o = np.arange(S) // 50
    q_full = np.empty((N, C), np.float32)
    for d in range(M):
        gidx = jt_of_o * 128 + 16 * d + pp_of_o
        q_full[gidx] = res.results[d]["q_out"].transpose(1, 0, 2).reshape(S, C)
    return np.ascontiguousarray(q_full.reshape(1, H, W, C), dtype=np.float32)



# revision 7
# speedup vs baseline: 1.1631x; 1.1631x over previous
"""CRF-RNN layer (nn_CrfRnnLayer) as an 8-core Trainium2 Bass kernel.

Distribution (sharding_hint): shard the N=H*W pixel dimension across the 8
cores; each core owns a strip of S=N/8=800 output pixels and holds the
(N x S) slices of both Gaussian kernels fully resident in SBUF as fp8.

The end-to-end metric here is dominated by the host->device tunnel
(~10ms/MB plus per-tensor overhead), so each core receives exactly ONE
small packed f16 tensor (~88KB) holding only its own strip of data:

  * u strip f16 (the f32 precision of u only matters for the final
    q = u - pairwise, which is reconstructed on the host from exact f32 u
    and the device's f16 `-pairwise` output),
  * bilateral feature rows (hi/lo fp16 split of [pos/alpha, rgb/beta]
    augmented with -|f|^2/2) in two orders: j-tile-major for the g-side
    (AllGathered on device into the full (21, N) lhsT feature matrix) and
    strip-major for the h-side,
  * static spatial feature rows, the exact separable spatial norm row, and
    the host-premultiplied compat*kernel weight matrix.

Phase 0 on device: AllGather the 16 g-order feature rows (25.6KB/core over
NeuronLink), assemble the full (21, N)/(6, N) g matrices with transposing
DMAs, then build both Gaussian kernel slices as fp8 in SBUF via fp16
matmuls + Exp, with norms via fp8 DoubleRow matmuls against ones.  The
iteration-1 softmax of u is computed on device and its AllGather overlaps
the kernel build.

Phase 1: 5 mean-field iterations (unchanged from the tuned baseline):
local softmax -> fp8 AllGather (padded to 32 cols) -> fp8 DoubleRow filter
matmuls against SBUF-resident kernels -> norm fold -> compatibility fold
via small matmuls -> q update.  The last iteration emits -pairwise as f16.
"""

import json

import numpy as np

from concourse import bacc, bass, mybir, tile
from concourse.bass_utils import run_bass_kernel_spmd

H = W = 80
C = 21
CP = 32              # padded class dim (fp8 DoubleRow needs 16B-aligned strides)
N = H * W            # 6400
M = 8                # cores
S = N // M           # 800 pixels per strip
NIT = 5
NS2 = N // 256       # 25 super-tiles of 256 pixels (DoubleRow pairs)
NJ = N // 128        # 50 j-tiles
PPC = 16             # partition slots owned per core within a j-tile
THETA_ALPHA, THETA_BETA, THETA_GAMMA = 160.0, 3.0, 3.0
F32 = mybir.dt.float32
F16 = mybir.dt.float16
FP8 = mybir.dt.float8e4
H1 = 512             # psum-bank split of the 800-wide strip

# packed-input layout (f16 elements)
# feature rows: fb_hi(5), -sq_hi, fb_lo(5), -sq_lo, 4 spatial, ones, zeros
# (ones/zeros rows ride along because vector memset needs 32-aligned
# partition bases; DMA-copied constant rows have no such constraint)
NFEAT = 18
U_LEN = 100 * 8 * C  # 16800
G_OFF = U_LEN                    # g-order feature rows, col = jt*16+pp
H_OFF = G_OFF + NFEAT * S        # h-order feature rows, col = o = pp*50+jt
R_OFF = H_OFF + NFEAT * S        # -1/spatial_norm strip row (o-order)
K_OFF = R_OFF + S                # 53x21 premultiplied weights
PK_LEN = K_OFF + 53 * C          # 44313

_CACHE = {}


def _split_bir_multiwaits(bir_json: bytes) -> bytes:
    """Split >1-sync-wait instructions into single-wait chains.

    The staged walrus build allows only one embedded sync-wait per
    instruction; prepend pure-wait EventSemaphores (same engine, same
    block) for all but the last wait.  Tile completion semaphores only
    count up within the kernel epoch, so waiting sequentially is
    equivalent to the simultaneous multi-wait.
    """
    d = json.loads(bir_json)
    for fn in d.get("functions", []):
        for blk in fn.get("blocks", []):
            out = []
            for inst in blk.get("instructions", []):
                si = inst.get("sync_info") or {}
                waits = si.get("on_wait") or []
                if len(waits) > 1:
                    for j, w in enumerate(waits[:-1]):
                        out.append({
                            "debug": inst.get("debug", 0),
                            "engine": inst["engine"],
                            "ins": [],
                            "name": f"{inst['name']}-sw{j}",
                            "opcode": "EventSemaphore",
                            "outs": [],
                            "sync_info": {"on_update": [], "on_wait": [w]},
                        })
                    si["on_wait"] = [waits[-1]]
                out.append(inst)
            blk["instructions"] = out
    return json.dumps(d).encode()


def _install_birpatch():
    if _CACHE.get("birpatch"):
        return
    from concourse import bass2jax
    orig = bass2jax.compile_bir_kernel

    def patched(bir_json, tmpdir, neff_name="file.neff"):
        return orig(_split_bir_multiwaits(bir_json), tmpdir, neff_name)

    bass2jax.compile_bir_kernel = patched
    _CACHE["birpatch"] = True


def _build_program(nit=NIT):
    key = ("nc", nit)
    if key in _CACHE:
        return _CACHE[key]
    nc = bacc.Bacc("TRN2", target_bir_lowering=False, debug=False, num_devices=M)

    pk = nc.dram_tensor("pk", [PK_LEN], F16, kind="ExternalInput")
    pw_out = nc.dram_tensor("pw_out", [100, 8, C], F16, kind="ExternalOutput")

    EXP = mybir.ActivationFunctionType.Exp
    COPY = mybir.ActivationFunctionType.Copy
    DR = mybir.MatmulPerfMode.DoubleRow

    def pkrows(off, r0, r1):
        return pk[off + r0 * S: off + r1 * S].rearrange("(f s) -> f s", f=r1 - r0)

    with tile.TileContext(nc) as tc:
        with (
            tc.tile_pool(name="const", bufs=1) as constp,
            tc.tile_pool(name="smtile", bufs=2) as smtp,
            tc.tile_pool(name="smfull", bufs=2) as smfp,
            tc.tile_pool(name="fcopy", bufs=2) as fcp,
            tc.tile_pool(name="qpool", bufs=2) as qp,
            tc.tile_pool(name="dram_cc", bufs=2, space="DRAM") as dramcc,
            tc.tile_pool(name="dram_ft", bufs=1, space="DRAM") as dramft,
        ):
            # ---- stage + AllGather the g-order feature rows (must be the
            # first collective: phase 0 depends on it) ----
            stg = dramft.tile([NFEAT, S], F16, tag="stg")
            nc.sync.dma_start(stg[:, :], pkrows(G_OFF, 0, NFEAT))
            gth = dramft.tile([M * NFEAT, S], F16, tag="gth")
            nc.gpsimd.collective_compute(
                "AllGather",
                mybir.AluOpType.bypass,
                replica_groups=[list(range(M))],
                ins=[stg[:, :].opt()],
                outs=[gth[:, :].opt()],
            )

            # ---- unpack strip-local sections ----
            u16 = constp.tile([100, 8, C], F16, tag="u16")
            nc.sync.dma_start(
                u16[:], pk[0:U_LEN].rearrange("(i j c) -> i j c", i=100, j=8))
            u_sb = constp.tile([100, 8, C], F32, tag="u")
            nc.scalar.activation(u_sb[:], u16[:], COPY)

            hb_sb = constp.tile([C, S], F16, tag="hb")
            nc.sync.dma_start(hb_sb[0:6, :], pkrows(H_OFF, 0, 6))
            nc.sync.dma_start(hb_sb[6:7, :], pkrows(H_OFF, 16, 17))
            nc.sync.dma_start(hb_sb[7:13, :], pkrows(H_OFF, 6, 12))
            nc.sync.dma_start(hb_sb[13:14, :], pkrows(H_OFF, 17, 18))
            nc.sync.dma_start(hb_sb[14:20, :], pkrows(H_OFF, 0, 6))
            nc.sync.dma_start(hb_sb[20:21, :], pkrows(H_OFF, 16, 17))

            hs_sb = constp.tile([6, S], F16, tag="hs")
            nc.sync.dma_start(hs_sb[0:1, :], pkrows(H_OFF, 16, 17))
            nc.sync.dma_start(hs_sb[1:2, :], pkrows(H_OFF, 16, 17))
            nc.sync.dma_start(hs_sb[2:6, :], pkrows(H_OFF, 12, 16))

            rs_row = constp.tile([1, S], F16, tag="rsrow")
            nc.sync.dma_start(rs_row[:], pkrows(R_OFF, 0, 1))
            skbk_sb = constp.tile([53, C], F16, tag="skbk")
            nc.sync.dma_start(
                skbk_sb[:],
                pk[K_OFF:K_OFF + 53 * C].rearrange("(f c) -> f c", f=53))

            ones2 = constp.tile([128, 2, 16], FP8, tag="ones2")
            nc.vector.memset(ones2[:], 1.0)
            onesC = constp.tile([1, C], F16, tag="onesC")
            nc.vector.memset(onesC[:], 1.0)

            kb_sb = constp.tile([128, NS2, 2, S], FP8, tag="kb")
            ks_sb = constp.tile([128, NS2, 2, S], FP8, tag="ks")
            # broadcast -1/norm rows, (C, S) per kernel
            rbc_sb = constp.tile([C, S], F32, tag="rbc")
            rsc_sb = constp.tile([C, S], F32, tag="rsc")

            X = mybir.AxisListType.X
            ADD = mybir.AluOpType.add

            def softmax_and_gather(q_tile):
                smcat = smtp.tile([100, 8, CP], FP8, tag="smcat")
                nc.vector.memset(smcat[:, :, C:CP], 0.0)
                esb = smtp.tile([100, 8, C], F32, tag="esb")
                nc.scalar.activation(esb[:], q_tile[:], EXP)
                ssum = smtp.tile([100, 8], F32, tag="ssum")
                nc.vector.tensor_reduce(ssum[:], esb[:], X, ADD)
                rsum = smtp.tile([100, 8], F32, tag="rsum")
                nc.vector.reciprocal(rsum[:], ssum[:])
                for s8 in range(8):
                    nc.vector.tensor_scalar_mul(
                        smcat[:, s8, 0:C], esb[:, s8, :], rsum[:, s8:s8 + 1])
                sm_in = dramcc.tile([S, CP], FP8, tag="sm_in")
                nc.sync.dma_start(
                    sm_in[:, :].rearrange("(s p) c -> p s c", p=100), smcat[:])
                sm_all = dramcc.tile([N, CP], FP8, tag="sm_all")
                nc.gpsimd.collective_compute(
                    "AllGather",
                    mybir.AluOpType.bypass,
                    replica_groups=[list(range(M))],
                    ins=[sm_in[:, :].opt()],
                    outs=[sm_all[:, :].opt()],
                )
                return sm_all

            # iteration-1 softmax of u: issued second so its AllGather
            # overlaps the phase-0 kernel build
            sm_all = softmax_and_gather(u_sb)

            # ---- assemble full (21, N)/(6, N) g matrices from the gather:
            # gathered row (d, f), col (jt, pp)  ->  g[f, jt*128 + 16d + pp]
            gap = gth[:, :].rearrange(
                "(d f) (j p) -> f j d p", d=M, f=NFEAT, j=NJ, p=PPC)
            gb_sb = constp.tile([C, N], F16, tag="gb")
            gbv = gb_sb[:].rearrange("f (j d p) -> f j d p", j=NJ, d=M, p=PPC)
            gs_sb = constp.tile([6, N], F16, tag="gs")
            gsv = gs_sb[:].rearrange("f (j d p) -> f j d p", j=NJ, d=M, p=PPC)
            nc.sync.dma_start(gbv[0:5], gap[0:5])       # fb_hi
            nc.sync.dma_start(gbv[5:6], gap[16:17])     # ones
            nc.sync.dma_start(gbv[6:7], gap[5:6])       # -sq_hi
            nc.sync.dma_start(gbv[7:12], gap[0:5])      # fb_hi
            nc.sync.dma_start(gbv[12:13], gap[16:17])   # ones
            nc.sync.dma_start(gbv[13:14], gap[5:6])     # -sq_hi
            nc.sync.dma_start(gbv[14:19], gap[6:11])    # fb_lo
            nc.sync.dma_start(gbv[19:20], gap[17:18])   # zeros
            nc.sync.dma_start(gbv[20:21], gap[11:12])   # -sq_lo
            nc.sync.dma_start(gsv[0:2], gap[12:14])     # x'^2, y'^2
            nc.sync.dma_start(gsv[2:3], gap[16:17])     # ones
            nc.sync.dma_start(gsv[3:4], gap[16:17])     # ones
            nc.sync.dma_start(gsv[4:6], gap[14:16])     # 2x', 2y'

            # ---- phase 0: materialize fp8 kernel slices in SBUF + norms ----
            with (
                tc.tile_pool(name="psum_ip", bufs=2, space="PSUM") as pip,
                tc.tile_pool(name="psum_norm", bufs=1, space="PSUM") as pnorm,
                tc.tile_pool(name="psum_bc", bufs=1, space="PSUM") as pbc,
            ):
                for g_sb, h_sb, k_sb, scale, with_norm in (
                    (gb_sb, hb_sb, kb_sb, 1.0, True),
                    (gs_sb, hs_sb, ks_sb, -1.0 / 18.0, False),
                ):
                    if with_norm:
                        norm_ps = pnorm.tile([1, S], F32, tag="norm")
                    for st in range(NS2):
                        for t2 in range(2):
                            T = st * 2 + t2
                            ip = pip.tile([128, S], F32, tag="ip")
                            nc.tensor.matmul(
                                ip[:, 0:H1],
                                lhsT=g_sb[:, T * 128:(T + 1) * 128],
                                rhs=h_sb[:, 0:H1], start=True, stop=True)
                            nc.tensor.matmul(
                                ip[:, H1:S],
                                lhsT=g_sb[:, T * 128:(T + 1) * 128],
                                rhs=h_sb[:, H1:S], start=True, stop=True)
                            nc.scalar.activation(
                                k_sb[:, st, t2, :], ip[:, :], EXP, scale=scale)
                        if with_norm:
                            # norm accumulation rides the PE gaps behind Exp
                            nc.tensor.matmul(
                                norm_ps[0:1, 0:H1], lhsT=ones2[:, :, 0:1],
                                rhs=k_sb[:, st, :, 0:H1],
                                start=(st == 0), stop=(st == NS2 - 1),
                                perf_mode=DR)
                            nc.tensor.matmul(
                                norm_ps[0:1, H1:S], lhsT=ones2[:, :, 0:1],
                                rhs=k_sb[:, st, :, H1:S],
                                start=(st == 0), stop=(st == NS2 - 1),
                                perf_mode=DR)
                    if not with_norm:
                        continue
                    # -1/norm broadcast to all C partitions:
                    # reciprocal -> negate (fp16) -> K=1 matmul broadcast
                    nr_sb = smtp.tile([1, S], F32, tag="nr")
                    nc.vector.reciprocal(nr_sb[:], norm_ps[0:1, :])
                    nr16 = smtp.tile([1, S], F16, tag="nr16")
                    nc.scalar.activation(nr16[:], nr_sb[:], COPY, scale=-1.0)
                    bc_ps = pbc.tile([C, S], F32, tag="bc")
                    nc.tensor.matmul(bc_ps[:, 0:H1], lhsT=onesC[:],
                                     rhs=nr16[0:1, 0:H1], start=True, stop=True)
                    nc.tensor.matmul(bc_ps[:, H1:S], lhsT=onesC[:],
                                     rhs=nr16[0:1, H1:S], start=True, stop=True)
                    nc.vector.tensor_copy(rbc_sb[:], bc_ps[:, :])
                # spatial -1/norm comes precomputed (exact separable sums):
                # broadcast the f16 row to all C partitions the same way
                bc2_ps = pbc.tile([C, S], F32, tag="bc")
                nc.tensor.matmul(bc2_ps[:, 0:H1], lhsT=onesC[:],
                                 rhs=rs_row[0:1, 0:H1], start=True, stop=True)
                nc.tensor.matmul(bc2_ps[:, H1:S], lhsT=onesC[:],
                                 rhs=rs_row[0:1, H1:S], start=True, stop=True)
                nc.vector.tensor_copy(rsc_sb[:], bc2_ps[:, :])

            # ---- phase 1: mean-field iterations ----
            with (
                tc.tile_pool(name="psum_acc", bufs=1, space="PSUM") as pacc,
                tc.tile_pool(name="psum_pw", bufs=2, space="PSUM") as ppw,
            ):
                for it in range(nit):
                    smf = smfp.tile([128, NS2, 2, CP], FP8, tag="smf")
                    nc.sync.dma_start(
                        smf[:],
                        sm_all[:, :].rearrange("(p s t) c -> p s t c",
                                               s=NS2, t=2))
                    psb = pacc.tile([C, S], F32, tag="psb")
                    pss = pacc.tile([C, S], F32, tag="pss")
                    for st in range(NS2):
                        lhs = smf[:, st, :, 0:C]
                        st_f, sp_f = (st == 0), (st == NS2 - 1)
                        nc.tensor.matmul(psb[:, 0:H1], lhsT=lhs,
                                         rhs=kb_sb[:, st, :, 0:H1],
                                         start=st_f, stop=sp_f, perf_mode=DR)
                        nc.tensor.matmul(psb[:, H1:S], lhsT=lhs,
                                         rhs=kb_sb[:, st, :, H1:S],
                                         start=st_f, stop=sp_f, perf_mode=DR)
                        nc.tensor.matmul(pss[:, 0:H1], lhsT=lhs,
                                         rhs=ks_sb[:, st, :, 0:H1],
                                         start=st_f, stop=sp_f, perf_mode=DR)
                        nc.tensor.matmul(pss[:, H1:S], lhsT=lhs,
                                         rhs=ks_sb[:, st, :, H1:S],
                                         start=st_f, stop=sp_f, perf_mode=DR)
                    # normalized filter outputs -> stacked fp16 pairwise lhsT:
                    # rows 0:21 spatial*(-1/norm_s), rows 32:53 bilateral*(-1/norm_b)
                    fsb = fcp.tile([53, S], F16, tag="fsb")
                    nc.vector.memset(fsb[0:32, :], 0.0)
                    nc.vector.tensor_mul(fsb[0:C, 0:H1], pss[:, 0:H1],
                                         rsc_sb[:, 0:H1])
                    nc.vector.tensor_mul(fsb[0:C, H1:S], pss[:, H1:S],
                                         rsc_sb[:, H1:S])
                    nc.vector.tensor_mul(fsb[32:53, 0:H1], psb[:, 0:H1],
                                         rbc_sb[:, 0:H1])
                    nc.vector.tensor_mul(fsb[32:53, H1:S], psb[:, H1:S],
                                         rbc_sb[:, H1:S])
                    # pairwise fold: one 42-dim matmul per pixel block
                    pw = ppw.tile([100, 8, C], F32, tag="pw")
                    for s8 in range(8):
                        sl = slice(s8 * 100, (s8 + 1) * 100)
                        nc.tensor.matmul(pw[:, s8, :], lhsT=fsb[:, sl],
                                         rhs=skbk_sb[:], start=True, stop=True)
                    if it < nit - 1:
                        qn = qp.tile([100, 8, C], F32, tag="qn")
                        nc.vector.tensor_add(qn[:], u_sb[:], pw[:, :, :])
                        sm_all = softmax_and_gather(qn)
                    else:
                        # emit -pairwise only; the host adds exact f32 u
                        pw16 = qp.tile([100, 8, C], F16, tag="pw16")
                        nc.scalar.activation(pw16[:], pw[:, :, :], COPY)
                        nc.sync.dma_start(pw_out[:, :, :], pw16[:])

    nc.compile()
    _CACHE[key] = nc
    return nc


def _static_prep():
    """Input-independent host data: index maps and the static pk sections."""
    st = _CACHE.get("static")
    if st is not None:
        return st
    o = np.arange(S)
    jt_of_o, pp_of_o = o % NJ, o // NJ           # o = pp*50 + jt
    k = np.arange(S)
    o_of_k = (k % PPC) * NJ + k // PPC           # g-order col k = jt*16+pp
    # global pixel index per (core, strip pos): px = jt*128 + 16d + pp
    gidx = np.empty((M, S), np.int64)
    for d in range(M):
        gidx[d] = jt_of_o * 128 + PPC * d + pp_of_o
    gidxG = gidx[:, o_of_k]                      # g-order global indices

    ys, xs = np.meshgrid(np.arange(H, dtype=np.float32),
                         np.arange(W, dtype=np.float32), indexing="ij")
    xs, ys = xs.ravel(), ys.ravel()
    xi, yi = xs - 40.0, ys - 40.0                # centered, f16-exact ints

    # exact spatial norm row (separable full 1D sums)
    dx = np.arange(W, dtype=np.float64)
    kx1 = np.exp(-((dx[:, None] - dx[None, :]) ** 2) / 18.0)
    dyv = np.arange(H, dtype=np.float64)
    ky1 = np.exp(-((dyv[:, None] - dyv[None, :]) ** 2) / 18.0)
    norm_s = (ky1.sum(axis=1)[:, None] * kx1.sum(axis=1)[None, :]).ravel()
    rsc_neg = (-1.0 / norm_s).astype(np.float16)

    pk_all = np.zeros((M, PK_LEN), np.float16)
    for d in range(M):
        gsec = pk_all[d, G_OFF:H_OFF].reshape(NFEAT, S)
        gsec[12] = (xi * xi)[gidxG[d]]
        gsec[13] = (yi * yi)[gidxG[d]]
        gsec[14] = (2.0 * xi)[gidxG[d]]
        gsec[15] = (2.0 * yi)[gidxG[d]]
        gsec[16] = 1.0
        hsec = pk_all[d, H_OFF:R_OFF].reshape(NFEAT, S)
        hsec[12] = (xi * xi)[gidx[d]]
        hsec[13] = (yi * yi)[gidx[d]]
        hsec[14] = (-xi)[gidx[d]]
        hsec[15] = (-yi)[gidx[d]]
        hsec[16] = 1.0
        pk_all[d, R_OFF:K_OFF] = rsc_neg[gidx[d]]

    st = {"gidx": gidx, "gidxG": gidxG, "xs": xs, "ys": ys, "pk_all": pk_all}
    _CACHE["static"] = st
    return st


def _host_prep(unaries, rgb, spatial_kernel, bilateral_kernel,
               compatibility_matrix):
    st = _static_prep()
    gidx, gidxG, pk_all = st["gidx"], st["gidxG"], st["pk_all"]

    u_cn = np.ascontiguousarray(unaries, dtype=np.float32)[0].reshape(N, C)
    img = np.ascontiguousarray(rgb, dtype=np.float32)[0].reshape(N, 3)
    sk = np.asarray(spatial_kernel, dtype=np.float32)
    bk = np.asarray(bilateral_kernel, dtype=np.float32)
    cm = np.asarray(compatibility_matrix, dtype=np.float32)

    # bilateral feature rows: hi/lo fp16 split of [fb, -0.5|fb|^2]
    fb = np.empty((6, N), np.float32)
    fb[0] = st["xs"] / THETA_ALPHA
    fb[1] = st["ys"] / THETA_ALPHA
    fb[2:5] = img.T / THETA_BETA
    fb[5] = -0.5 * (fb[0] ** 2 + fb[1] ** 2 + fb[2] ** 2 + fb[3] ** 2
                    + fb[4] ** 2)
    hi = fb.astype(np.float16)
    lo = (fb - hi.astype(np.float32)).astype(np.float16)
    big12 = np.concatenate([hi, lo], axis=0)     # rows: fb_hi,-sq_hi,fb_lo,-sq_lo

    # u strips: (core, o) -> (core, i=o%100, j=o//100, c)
    u_strips = u_cn[gidx].reshape(M, 8, 100, C).transpose(0, 2, 1, 3)
    pk_all[:, 0:U_LEN] = u_strips.reshape(M, U_LEN).astype(np.float16)
    # bilateral rows in both orders
    pk_g = pk_all[:, G_OFF:H_OFF].reshape(M, NFEAT, S)
    pk_g[:, 0:12] = big12[:, gidxG].transpose(1, 0, 2)
    pk_h = pk_all[:, H_OFF:R_OFF].reshape(M, NFEAT, S)
    pk_h[:, 0:12] = big12[:, gidx].transpose(1, 0, 2)
    # premultiplied weights with the PE-quadrant zero gap
    skbk = np.zeros((53, C), np.float16)
    skbk[0:C] = (cm @ sk).T
    skbk[32:53] = (cm @ bk).T
    pk_all[:, K_OFF:PK_LEN] = skbk.reshape(-1)

    return [{"pk": pk_all[d]} for d in range(M)], u_cn, gidx


def kernel(unaries, rgb, spatial_kernel, bilateral_kernel,
           compatibility_matrix, _run_kwargs=None):
    _install_birpatch()
    nc = _build_program()
    in_maps, u_cn, gidx = _host_prep(unaries, rgb, spatial_kernel,
                                     bilateral_kernel, compatibility_matrix)
    kwargs = dict(_run_kwargs or {})
    res = run_bass_kernel_spmd(nc, in_maps, core_ids=list(range(M)), **kwargs)
    _CACHE["last_results"] = res
    q_full = np.empty((N, C), np.float32)
    for d in range(M):
        pw = res.results[d]["pw_out"].astype(np.float32)   # (100, 8, C) = -pairwise
        q_full[gidx[d]] = u_cn[gidx[d]] + pw.transpose(1, 0, 2).reshape(S, C)
    return np.ascontiguousarray(q_full.reshape(1, H, W, C), dtype=np.float32)
